# revision 18
# baseline (speedup 1.0000x reference)
"""Trainium2 Bass kernel v4 for nn_Conv1dMultiscaleLocalization.

Per image [768,768], one image per core (B=8 data-parallel):
  resp_j = vconv(C, k_j) + hconv(S, k_j);  conv = max_j resp_j
  pooled = 11x11 max pool; mask = (conv==mw(conv)) & (conv>0.5)

v4 vs v2 (trace-driven redesign):
  - Conv matmuls in fp8 e4m3 with DoubleRow perf mode (contraction 256 =
    64 rows x 4 residual terms, 0.5 cyc/out-elem): ~2x PE throughput vs
    the bf16 hi/lo scheme.  Terms are scaled residuals (2^0,2^-4,2^-6,
    2^-6); weights are exact +-2^-m bands so every product is exact and
    the DR pair-adder sees only small-mantissa operands close in
    exponent (pair-sums exact; verified on HW).
  - s0, s1 prefolded into the j0/j1 fp8 weights (still exact powers of
    two), so the combine is one pure TT pair-max + five ACT scaled
    copies (scalar engine) + a DVE TT max chain.  posg = true conv.
  - Mask fused to a single STT: (plv max nextafter(0.5)) is_le posg.
  (gpsimd 2-input elementwise ops are rejected by the walrus verifier
   on this toolchain, so all TT/STT stay on DVE.)
"""
import sys
import numpy as np

sys.path.insert(0, "/opt/trn_rl_repo")

import ml_dtypes  # noqa: E402
import concourse.bacc as bacc  # noqa: E402
import concourse.mybir as mybir  # noqa: E402
import concourse.tile as tile  # noqa: E402
from concourse.bass_utils import run_bass_kernel_spmd  # noqa: E402

F32 = mybir.dt.float32
FP8 = mybir.dt.float8e4
U8 = mybir.dt.uint8
AF = mybir.ActivationFunctionType
ALU = mybir.AluOpType
DR = mybir.MatmulPerfMode.DoubleRow
E4 = ml_dtypes.float8_e4m3

H = W = 768
KERNEL_SIZES = [3, 9, 15, 21, 31, 51, 65]
NJ = 7
XJ = [(w - 1) // 2 for w in KERNEL_SIZES]
SC = [np.float32(1.0) / np.float32(w - 1) for w in KERNEL_SIZES]
NB = 6          # 128-row blocks per image
NEG = -3.0e38
MT = [0, 4, 6, 6]   # per-term scale exponents (data stored * 2^m, weight 2^-m)
# per-j weight prefold: s0=2^-1 and s1=2^-3 are exact in e4m3 even after
# the 2^-MT[t] term scaling; the other scales ride the combine stage.
GJ = [float(SC[0]), float(SC[1]), 1.0, 1.0, 1.0, 1.0, 1.0]
JG = [(0, 3), (3, 5), (5, 7)]
C0 = float(np.nextafter(np.float32(0.5), np.float32(1.0)))  # >0.5 threshold

_CACHE = {}


# ---------------------------------------------------------------- constants
def _band(d, x):
    return np.where((d >= -x) & (d <= -1), 1.0,
                    np.where((d >= 1) & (d <= x), -1.0, 0.0)).astype(np.float32)


def _term_of(p, kt):
    # (partition, ktile) -> term index; pairs (kt=0,1) at one partition are
    # consecutive terms so the DR pair-adder sees 2^4-separated operands.
    return 2 * (p % 2) + kt


def _tvdr():
    """V stationary [128, NJ, 3, 2, 128] fp8: chunk c covers input rows
    128b-32+64c+p//2; tv[p,j,c,kt,m] = band_j(v - m) * 2^-MT[t]."""
    T = np.zeros((128, NJ, 3, 2, 128), dtype=np.float32)
    p = np.arange(128)
    m = np.arange(128)
    for j in range(NJ):
        for c in range(3):
            v = -32 + 64 * c + p // 2          # [128]
            d = v[:, None] - m[None, :]        # [128,128]
            b = _band(d, XJ[j])
            for kt in range(2):
                t = _term_of(p, kt)            # [128]
                T[:, j, c, kt, :] = (b * GJ[j]
                                     * (2.0 ** (-np.asarray(MT)[t]))[:, None])
    return T.astype(E4)


def _thdr():
    """H moving [128, 2, NJ, 128] fp8: out col w = 64a - 32 + n;
    th[p,kt,j,n] = band_j(p//2 + 32 - n) * 2^-MT[t]."""
    T = np.zeros((128, 2, NJ, 128), dtype=np.float32)
    p = np.arange(128)
    n = np.arange(128)
    for j in range(NJ):
        d = (p // 2)[:, None] + 32 - n[None, :]
        b = _band(d, XJ[j])
        for kt in range(2):
            t = _term_of(p, kt)
            T[:, kt, j, :] = (b * GJ[j]
                              * (2.0 ** (-np.asarray(MT)[t]))[:, None])
    return T.astype(E4)


def _split4(x):
    """x [H,W] f32 -> [4,H,W] e4m3 planes of scaled residuals."""
    planes = np.empty((4,) + x.shape, dtype=E4)
    r = x.astype(np.float32)
    for t, m in enumerate(MT):
        q = (r * np.float32(2.0 ** m)).astype(E4)
        planes[t] = q
        r = r - q.astype(np.float32) * np.float32(2.0 ** (-m))
    return planes


def _prep_core(Cb, Sb):
    """cdr [128, NB, 3, 2, W] fp8; sdr [128, 12, 2, NB, 128] fp8."""
    p = np.arange(128)
    kt = np.arange(2)
    t_pk = 2 * (p[:, None] % 2) + kt[None, :]          # [128,2]

    P4 = _split4(Cb)                                   # [4,768,768]
    P4p = np.zeros((4, 832, W), dtype=E4)
    P4p[:, 32:800, :] = P4
    # rows: v+32 = 128*ib + 64*c + p//2
    ib = np.arange(NB)
    c3 = np.arange(3)
    rows = (128 * ib[:, None, None] + 64 * c3[None, :, None]
            + (p // 2)[None, None, :])                 # [NB,3,128]
    # cdr[p, ib, c, kt, n]
    cdr = P4p[t_pk[:, None, None, :], rows.transpose(2, 0, 1)[:, :, :, None], :]
    cdr = np.ascontiguousarray(cdr)                    # [128,NB,3,2,W] e4m3

    S4 = _split4(np.ascontiguousarray(Sb.T))           # [4,768(w'),768(u)]
    a12 = np.arange(12)
    wrow = 64 * a12[:, None] + (p // 2)[None, :]       # [12,128]
    # sdr[p, a, kt, ub, m] = S4[t(p,kt), w'(a,p), 128*ub+m]
    sdr = S4[t_pk[:, None, :], wrow.T[:, :, None], :]  # [128,12,2,768]
    sdr = sdr.reshape(128, 12, 2, NB, 128)
    return {"cdr": cdr.reshape(128, -1), "sdr": np.ascontiguousarray(sdr).reshape(128, -1)}


def _consts():
    return {
        "tvdr": np.ascontiguousarray(_tvdr()).reshape(128, -1),
        "thdr": np.ascontiguousarray(_thdr()).reshape(128, -1),
        "idt": np.eye(128, dtype=np.float32),
    }


# ---------------------------------------------------------------- kernel IR
def _build():
    nc = bacc.Bacc()
    CDRD = nc.declare_dram_parameter("cdr", [128, NB * 3 * 2 * W], FP8,
                                     isOutput=False)
    SDRD = nc.declare_dram_parameter("sdr", [128, 12 * 2 * NB * 128], FP8,
                                     isOutput=False)
    TVD = nc.declare_dram_parameter("tvdr", [128, NJ * 3 * 2 * 128], FP8,
                                    isOutput=False)
    THD = nc.declare_dram_parameter("thdr", [128, 2 * NJ * 128], FP8,
                                    isOutput=False)
    IDT = nc.declare_dram_parameter("idt", [128, 128], F32, isOutput=False)
    CONV = nc.declare_dram_parameter("conv", [H, W], F32, isOutput=True)
    MASK = nc.declare_dram_parameter("mask", [H, W], U8, isOutput=True)

    def hspan(a, j):
        return max(0, 64 * a - XJ[j]), min(W, 64 * a + 64 + XJ[j])

    with tile.TileContext(nc) as tc:
        with tc.tile_pool(name="big", bufs=1) as big, \
             tc.tile_pool(name="consts", bufs=1) as cst, \
             tc.tile_pool(name="posg", bufs=1) as posp, \
             tc.tile_pool(name="pool", bufs=2) as poolp, \
             tc.tile_pool(name="atg", bufs=1) as atgp, \
             tc.tile_pool(name="ptv", bufs=2) as ptvp, \
             tc.tile_pool(name="tmp", bufs=2) as tmpp, \
             tc.tile_pool(name="small", bufs=2) as smallp, \
             tc.tile_pool(name="ps", bufs=3, space="PSUM") as ps, \
             tc.tile_pool(name="psT", bufs=2, space="PSUM") as psT:

            cdr = big.tile([128, NB, 3, 2, W], FP8, tag="cdr", name="cdr")
            sdr = big.tile([128, 12, 2, NB, 128], FP8, tag="sdr", name="sdr")
            tv = cst.tile([128, NJ, 3, 2, 128], FP8, tag="tv")
            th = cst.tile([128, 2, NJ, 128], FP8, tag="th")
            idt = cst.tile([128, 128], F32, tag="idt")

            # loads: first-wave critical path first (tv j0-2, cdr b0, th),
            # then the rest block-major on both queues.
            nc.sync.dma_start(out=tv[:, 0:3, :, :, :], in_=TVD[:, 0:3 * 768])
            nc.scalar.dma_start(out=th[:], in_=THD[:])
            nc.sync.dma_start(out=cdr[:, 0, :, :, :],
                              in_=CDRD[:, 0:3 * 2 * W])
            nc.scalar.dma_start(out=sdr[:], in_=SDRD[:])
            nc.sync.dma_start(out=tv[:, 3:NJ, :, :, :], in_=TVD[:, 3 * 768:])
            nc.scalar.dma_start(out=idt[:], in_=IDT[:])
            for b in range(1, NB):
                nc.sync.dma_start(out=cdr[:, b, :, :, :],
                                  in_=CDRD[:, b * 6 * W:(b + 1) * 6 * W])

            posg = [posp.tile([128, 800], F32, tag=f"posg{ib}",
                              name=f"posg{ib}") for ib in range(NB)]
            atg = atgp.tile([128, NB, 800], F32, tag="atg", name="atg")
            nc.vector.memset(atg[:, :, 0:16], NEG)
            nc.vector.memset(atg[:, :, 784:800], NEG)

            def emit_wave(ib):
                # combine: j0/j1 prefolded in the fp8 weights -> one pure TT
                # pair-max; j2..j6 evacuated via ACT scaled copies; DVE chain
                # merges them.  posg ends up in TRUE scale (= conv).
                cc = tmpp.tile([128, 768], F32, tag="cc", name="cc")
                us = {j: tmpp.tile([128, 768], F32, tag=f"u{j}", name=f"u{j}")
                      for j in range(1, 7)}
                g = posg[ib][:, 16:784]
                for wi, (j0, j1) in enumerate(JG):
                    ptiles = {j: ps.tile([128, 2, 384], F32, tag="p",
                                         name=f"p{j}",
                                         padded_shape=[128, 2, 512])
                              for j in range(j0, j1)}
                    # V: 3 DR chunks per j
                    for j in range(j0, j1):
                        for c in range(3):
                            for h in range(2):
                                nc.tensor.matmul(
                                    ptiles[j][:, h, :],
                                    tv[:, j, c, :, :],
                                    cdr[:, ib, c, :, 384 * h:384 * (h + 1)],
                                    start=(c == 0), stop=False,
                                    perf_mode=DR, skip_group_check=True)
                    # H: stationary = sdr chunk (a); moving = th band slices
                    spans = {}
                    for a in range(12):
                        for j in range(j0, j1):
                            lo, hi = hspan(a, j)
                            if lo < hi:
                                spans[(a, j)] = (lo, hi)
                    lasts = {}
                    for (a, j) in spans:
                        lasts[j] = a
                    for a in range(12):
                        if not any((a, j) in spans for j in range(j0, j1)):
                            continue
                        lhs = sdr[:, a, :, ib, :]
                        for j in range(j0, j1):
                            if (a, j) not in spans:
                                continue
                            lo, hi = spans[(a, j)]
                            for h in range(2):
                                l2 = max(lo, 384 * h)
                                h2 = min(hi, 384 * (h + 1))
                                if l2 >= h2:
                                    continue
                                off = l2 - (64 * a - 32)
                                stop = (lasts[j] == a and h2 == hi)
                                nc.tensor.matmul(
                                    ptiles[j][:, h, l2 - 384 * h:h2 - 384 * h],
                                    lhs, th[:, :, j, off:off + (h2 - l2)],
                                    start=False, stop=stop,
                                    perf_mode=DR, skip_group_check=True)
                    if wi == 0:
                        # TT may read only ONE input from PSUM: evac p1 first
                        nc.scalar.activation(us[1][:], ptiles[1][:, :, :],
                                             AF.Copy)
                        nc.vector.tensor_tensor(
                            cc[:], ptiles[0][:, :, :], us[1][:], ALU.max)
                        nc.scalar.activation(us[2][:], ptiles[2][:, :, :],
                                             AF.Copy, scale=float(SC[2]))
                        nc.vector.tensor_tensor(cc[:], cc[:], us[2][:],
                                                ALU.max)
                    elif wi == 1:
                        for j in (3, 4):
                            nc.scalar.activation(us[j][:], ptiles[j][:, :, :],
                                                 AF.Copy, scale=float(SC[j]))
                        nc.vector.tensor_tensor(cc[:], cc[:], us[3][:],
                                                ALU.max)
                        nc.vector.tensor_tensor(cc[:], cc[:], us[4][:],
                                                ALU.max)
                    else:
                        for j in (5, 6):
                            nc.scalar.activation(us[j][:], ptiles[j][:, :, :],
                                                 AF.Copy, scale=float(SC[j]))
                        nc.vector.tensor_tensor(cc[:], cc[:], us[5][:],
                                                ALU.max)
                        nc.vector.tensor_tensor(g, cc[:], us[6][:], ALU.max)

            def emit_mwh(ib):
                nc.vector.memset(posg[ib][:, 0:16], NEG)
                nc.vector.memset(posg[ib][:, 784:800], NEG)
                m2 = poolp.tile([128, 800], F32, tag="m2", name="m2")
                m4 = poolp.tile([128, 800], F32, tag="m4", name="m4")
                m8 = poolp.tile([128, 800], F32, tag="m8", name="m8")
                a = poolp.tile([128, W], F32, tag="a", name="a")
                g = posg[ib]
                nc.vector.tensor_tensor(m2[:, 0:799], g[:, 0:799], g[:, 1:800],
                                        ALU.max)
                nc.vector.tensor_tensor(m4[:, 0:797], m2[:, 0:797],
                                        m2[:, 2:799], ALU.max)
                nc.vector.tensor_tensor(m8[:, 0:793], m4[:, 0:793],
                                        m4[:, 4:797], ALU.max)
                nc.vector.tensor_tensor(a[:], m8[:, 11:779], m4[:, 18:786],
                                        ALU.max)
                return a

            def emit_at(ib, a):
                for half in range(2):
                    pt = psT.tile([128, 384], F32, tag="pt", name="pt")
                    for k in range(3):
                        c = 3 * half + k
                        nc.tensor.transpose(pt[:, 128 * k:128 * (k + 1)],
                                            a[:, 128 * c:128 * (c + 1)],
                                            idt[:])
                    nc.scalar.activation(
                        atg[:, 3 * half:3 * half + 3,
                            16 + 128 * ib:16 + 128 * (ib + 1)],
                        pt[:], AF.Copy)

            def emit_mwv(vib):
                av = 16 + 128 * vib
                n = 128
                m2 = poolp.tile([128, NB, 144], F32, tag="m2v", name="m2v")
                m4 = poolp.tile([128, NB, 144], F32, tag="m4v", name="m4v")
                m8 = poolp.tile([128, NB, 144], F32, tag="m8v", name="m8v")
                pv = ptvp.tile([128, NB, 128], F32, tag="pv", name="pv")
                nc.vector.tensor_tensor(m2[:, :, 0:n + 16],
                                        atg[:, :, av - 8:av + n + 8],
                                        atg[:, :, av - 7:av + n + 9], ALU.max)
                nc.vector.tensor_tensor(m4[:, :, 0:n + 14], m2[:, :, 0:n + 14],
                                        m2[:, :, 2:n + 16], ALU.max)
                nc.vector.tensor_tensor(m8[:, :, 2:n + 10], m4[:, :, 2:n + 10],
                                        m4[:, :, 6:n + 14], ALU.max)
                nc.vector.tensor_tensor(pv[:, :, :], m8[:, :, 3:n + 3],
                                        m4[:, :, 10:n + 10], ALU.max)
                return pv

            def emit_ptt(vib, pv):
                plv = smallp.tile([128, W], F32, tag="plv", name="plv")
                for half in range(2):
                    pt = psT.tile([128, 384], F32, tag="pt", name="pt")
                    for k in range(3):
                        c = 3 * half + k
                        nc.tensor.transpose(
                            pt[:, 128 * k:128 * (k + 1)],
                            pv[:, c, :], idt[:])
                    nc.scalar.activation(plv[:, 384 * half:384 * (half + 1)],
                                         pt[:], AF.Copy)
                mk = smallp.tile([128, W], U8, tag="mk", name="mk")
                nc.vector.scalar_tensor_tensor(
                    mk[:], plv[:], C0, posg[vib][:, 16:784],
                    ALU.max, ALU.is_le)
                nc.sync.dma_start(out=MASK[128 * vib:128 * (vib + 1), :],
                                  in_=mk[:])

            alist = {}
            pvlist = {}
            for ib in range(NB):
                emit_wave(ib)
                nc.sync.dma_start(out=CONV[128 * ib:128 * (ib + 1), :],
                                  in_=posg[ib][:, 16:784])
                alist[ib] = emit_mwh(ib)
                emit_at(ib, alist[ib])
                if ib >= 1:
                    pvlist[ib - 1] = emit_mwv(ib - 1)
                    emit_ptt(ib - 1, pvlist[ib - 1])
            pvlist[NB - 1] = emit_mwv(NB - 1)
            emit_ptt(NB - 1, pvlist[NB - 1])

    nc.compile()
    return nc


# ---------------------------------------------------------------- host glue
def kernel(C, S, kernel_cos, kernel_sin):
    C = np.asarray(C, dtype=np.float32)
    S = np.asarray(S, dtype=np.float32)
    B = C.shape[0]
    if "nc" not in _CACHE:
        _CACHE["nc"] = _build()
    nc = _CACHE["nc"]
    consts = _consts()
    in_maps = []
    for b in range(B):
        m = _prep_core(C[b, 0], S[b, 0])
        m.update(consts)
        in_maps.append(m)
    res = run_bass_kernel_spmd(nc, in_maps, core_ids=list(range(B)))
    conv = np.stack([r["conv"] for r in res.results])[:, None]
    mask = np.stack([r["mask"] for r in res.results])[:, None].astype(bool)
    return conv.astype(np.float32), mask


# revision 21
# speedup vs baseline: 1.3564x; 1.3564x over previous
"""Trainium2 Bass kernel v2 for nn_Conv1dMultiscaleLocalization.

Per image [768,768], one image per core (B=8 data-parallel):
  resp_j = vconv(C, k_j) + hconv(S, k_j);  conv = max_j resp_j
  pooled = 11x11 max pool; mask = (conv==mw(conv)) & (conv>0.5)

Same numerics as v1 (bf16 hi+lo exact split, fp32 PSUM; 0 mask flips).
Perf changes vs v1 (trace-driven):
  - ldweights=False on consecutive same-stationary matmuls: H conv was
    LDWEIGHTS-gated (600 small matmuls each paying a ~116ns weight load);
    now ~1 load per stationary chunk.  Matmul order restructured (terms and
    halves grouped per stationary) to maximize reuse runs.
  - Batched DMA: c96 stays block-major; st becomes wave-major so each wave's
    H stationaries arrive in one [128,768] transfer; ~30 DMAs vs ~100.
  - mwv (vertical pool window) batched across all 6 column chunks per step
    via one 3D-AP instruction instead of 6, on a single atg tile.
"""
import sys
import numpy as np

sys.path.insert(0, "/opt/trn_rl_repo")

import ml_dtypes  # noqa: E402
import concourse.bacc as bacc  # noqa: E402
import concourse.mybir as mybir  # noqa: E402
import concourse.tile as tile  # noqa: E402
from concourse.bass_utils import run_bass_kernel_spmd  # noqa: E402

F32 = mybir.dt.float32
BF16 = mybir.dt.bfloat16
U8 = mybir.dt.uint8
AF = mybir.ActivationFunctionType
ALU = mybir.AluOpType

H = W = 768
KERNEL_SIZES = [3, 9, 15, 21, 31, 51, 65]
NJ = 7
XJ = [(w - 1) // 2 for w in KERNEL_SIZES]
SCALES = [1.0 / (w - 1) for w in KERNEL_SIZES]
NB = 6          # 128-row blocks per image
NEG = -3.0e38
NTERMS = 2      # bf16 split terms (hi, lo)
JG = [(0, 3), (3, 5), (5, 7)]
C0 = float(np.nextafter(np.float32(0.5), np.float32(1.0)))  # >0.5 threshold

_CACHE = {}


# ---------------------------------------------------------------- constants
def _sign_band(d, x):
    return np.where((d >= -x) & (d <= -1), 1.0,
                    np.where((d >= 1) & (d <= x), -1.0, 0.0))


def _toeplitz_v2():
    """V stationary [128, NJ*3, 128]: K packs (64 v-rows x 2 terms) by
    partition parity; chunk k covers input rows 128b-32+64k + p//2.
    T2[p, 3j+k, m] = band_j((-32 + 64k + p//2) - m)."""
    T = np.zeros((128, NJ * 3, 128), dtype=np.float32)
    p = np.arange(128)[:, None]
    m = np.arange(128)[None, :]
    for j in range(NJ):
        for k in range(3):
            T[:, 3 * j + k, :] = _sign_band((-32 + 64 * k + p // 2) - m, XJ[j])
    return T


def _band_h2():
    """H moving [128, NJ*128]: K packs (64 w'-cols x 2 terms); chunk a covers
    w' = 64a + p//2, out col w = 64a - 32 + n.
    T2[p, 128j+n] = band_j(p//2 + 32 - n)."""
    T = np.zeros((128, NJ * 128), dtype=np.float32)
    p = np.arange(128)[:, None]
    n = np.arange(128)[None, :]
    for j in range(NJ):
        T[:, 128 * j:128 * (j + 1)] = _sign_band(p // 2 + 32 - n, XJ[j])
    return T


def _split_terms(x):
    terms = []
    r = x
    for _ in range(NTERMS):
        t = r.astype(ml_dtypes.bfloat16)
        terms.append(t)
        r = r - t.astype(np.float32)
    return terms


def _interleave(t0, t1):
    """[R, ...] x2 -> [2R, ...] with rows (2r, 2r+1) = (t0[r], t1[r])."""
    out = np.empty((t0.shape[0] * 2,) + t0.shape[1:], dtype=t0.dtype)
    out[0::2] = t0
    out[1::2] = t1
    return out


def _prep_core(Cb, Sb):
    """c2 [128, NB*3, W]: c2[p, 3b+k, n] = term_{p%2}(C)[128b-32+64k+p//2, n];
    stw2 [128, NB, 12, 128]: stw2[p, ib, a, m] = term_{p%2}(S)[128ib+m,
    64a+p//2]."""
    ct = _split_terms(Cb)
    cint = _interleave(ct[0].astype(np.float32),
                       ct[1].astype(np.float32))  # [1536, W] rows 2v+t
    cpad = np.vstack([np.zeros((64, W), np.float32), cint,
                      np.zeros((192, W), np.float32)])  # row 2(v+32)+t
    c2 = np.zeros((128, NB * 3, W), dtype=np.float32)
    for b in range(NB):
        for k in range(3):
            r0 = 2 * (128 * b + 64 * k)  # = 2*(v0+32) with v0 = 128b-32+64k
            c2[:, 3 * b + k, :] = cpad[r0:r0 + 128, :]
    st = _split_terms(Sb.T)  # [w', u]
    sint = _interleave(st[0].astype(np.float32),
                       st[1].astype(np.float32))  # [1536, u] rows 2w'+t
    stw2 = sint.reshape(12, 128, NB, 128).transpose(1, 2, 0, 3)
    return {"c2": c2.astype(ml_dtypes.bfloat16).reshape(128, -1),
            "stw2": np.ascontiguousarray(stw2).astype(
                ml_dtypes.bfloat16).reshape(128, -1)}


def _consts():
    return {
        "TV2": _toeplitz_v2().astype(ml_dtypes.bfloat16).reshape(128, -1),
        "TH2": _band_h2().astype(ml_dtypes.bfloat16).reshape(128, -1),
        "IDT": np.eye(128, dtype=np.float32),
    }


# ---------------------------------------------------------------- kernel IR
def _build():
    nc = bacc.Bacc()
    C2D = nc.declare_dram_parameter("c2", [128, NB * 3 * W], BF16,
                                    isOutput=False)
    STW2 = nc.declare_dram_parameter("stw2", [128, NB * 12 * 128], BF16,
                                     isOutput=False)
    TV2D = nc.declare_dram_parameter("TV2", [128, NJ * 3 * 128], BF16,
                                     isOutput=False)
    TH2D = nc.declare_dram_parameter("TH2", [128, NJ * 128], BF16,
                                     isOutput=False)
    IDT = nc.declare_dram_parameter("IDT", [128, 128], F32, isOutput=False)
    CONV = nc.declare_dram_parameter("conv", [H, W], F32, isOutput=True)
    MASK = nc.declare_dram_parameter("mask", [H, W], U8, isOutput=True)

    def hspan(a, j):
        # out-col span covered by w'-chunk a (64 cols, both terms)
        return max(0, 64 * a - XJ[j]), min(W, 64 * a + 64 + XJ[j])

    with tile.TileContext(nc) as tc:
        with tc.tile_pool(name="big", bufs=1) as big, \
             tc.tile_pool(name="consts", bufs=1) as cst, \
             tc.tile_pool(name="posg", bufs=1) as posp, \
             tc.tile_pool(name="pool", bufs=2) as poolp, \
             tc.tile_pool(name="atg", bufs=1) as atgp, \
             tc.tile_pool(name="pooled", bufs=1) as pooledp, \
             tc.tile_pool(name="small", bufs=2) as smallp, \
             tc.tile_pool(name="ps", bufs=3, space="PSUM") as ps, \
             tc.tile_pool(name="psT", bufs=2, space="PSUM") as psT:

            c2 = big.tile([128, NB * 3, W], BF16, tag="c2", name="c2")
            stw2 = big.tile([128, NB, 12, 128], BF16, tag="stw2", name="stw2")
            tv2 = cst.tile([128, NJ * 3, 128], BF16, tag="tv2")
            th2 = cst.tile([128, NJ * 128], BF16, tag="th2")
            idt = cst.tile([128, 128], F32, tag="idt")
            # block-priority loads on BOTH hwdge queues: wave ib needs
            # c2[:, 3ib:3ib+3, :] (V) and stw2[:, ib, :, :] (H stationaries).
            # First-wave critical path: TV2 j0-2 slice, then c2 b0 per k-chunk.
            nc.sync.dma_start(out=tv2[:, 0:3, :], in_=TV2D[:, 0:3 * 128])
            nc.scalar.dma_start(out=th2[:], in_=TH2D[:])
            for k in range(3):
                nc.sync.dma_start(out=c2[:, k:k + 1, :],
                                  in_=C2D[:, k * W:(k + 1) * W])
            nc.sync.dma_start(out=tv2[:, 3:21, :], in_=TV2D[:, 3 * 128:])
            nc.scalar.dma_start(out=stw2[:, 0, :, :], in_=STW2[:, 0:12 * 128])
            nc.scalar.dma_start(out=idt[:], in_=IDT[:])
            for b in range(1, NB):
                nc.sync.dma_start(out=c2[:, 3 * b:3 * (b + 1), :],
                                  in_=C2D[:, 3 * b * W:3 * (b + 1) * W])
                nc.scalar.dma_start(
                    out=stw2[:, b, :, :],
                    in_=STW2[:, 12 * 128 * b:12 * 128 * (b + 1)])

            posg = [posp.tile([128, 800], F32, tag=f"posg{ib}", name=f"posg{ib}")
                    for ib in range(NB)]
            atg = atgp.tile([128, NB, 800], F32, tag="atg", name="atg")
            ptv = pooledp.tile([128, NB, W], F32, tag="ptv", name="ptv")
            nc.vector.memset(atg[:, :, 0:16], NEG)
            nc.vector.memset(atg[:, :, 784:800], NEG)

            def last_a(j, h):
                lo_h, hi_h = 384 * h, 384 * (h + 1)
                return max(a for a in range(12)
                           if max(hspan(a, j)[0], lo_h)
                           < min(hspan(a, j)[1], hi_h))

            def emit_wave(ib):
                for (j0, j1) in JG:
                    # 2-bank tiles: [128, 2, 384] padded to [128, 2, 512] so
                    # each half sits bank-aligned; combine reads both at once.
                    ptiles = {j: ps.tile([128, 2, 384], F32, tag="p",
                                         name=f"p{j}",
                                         padded_shape=[128, 2, 512])
                              for j in range(j0, j1)}
                    # ---- V: 3 K-chunks (64 v-rows x 2 terms each) per half
                    for j in range(j0, j1):
                        for k in range(3):
                            for h in range(2):
                                rhs = c2[:, 3 * ib + k, 384 * h:384 * (h + 1)]
                                nc.tensor.matmul(
                                    ptiles[j][:, h, :], tv2[:, 3 * j + k, :],
                                    rhs, start=(k == 0), stop=False,
                                    skip_group_check=True)
                    # ---- H: stationary = stw2 chunk (ib, a); both terms ride
                    spans = {}
                    for a in range(12):
                        for j in range(j0, j1):
                            lo, hi = hspan(a, j)
                            if lo < hi:
                                spans[(a, j)] = (lo, hi)
                    lasts = {}
                    for (a, j) in spans:
                        lasts[j] = a
                    for a in range(12):
                        if not any((a, j) in spans for j in range(j0, j1)):
                            continue
                        lhs = stw2[:, ib, a, :]
                        first = True
                        for j in range(j0, j1):
                            if (a, j) not in spans:
                                continue
                            lo, hi = spans[(a, j)]
                            for h in range(2):
                                l2 = max(lo, 384 * h)
                                h2 = min(hi, 384 * (h + 1))
                                if l2 >= h2:
                                    continue
                                off = 128 * j + (l2 - (64 * a - 32))
                                stop = (lasts[j] == a and h2 == hi)
                                mm = nc.tensor.matmul(
                                    ptiles[j][:, h, l2 - 384 * h:h2 - 384 * h],
                                    lhs, th2[:, off:off + (h2 - l2)],
                                    start=False, stop=stop,
                                    skip_group_check=True)
                                if not first:
                                    mm.ins.ldweights = False
                                first = False
                    # ---- combine into posg, 768 wide via the 2-bank AP
                    dst = posg[ib][:, 16:784]
                    for j in range(j0, j1):
                        if j == 0:
                            nc.scalar.activation(dst, ptiles[0][:, :, :],
                                                 AF.Copy,
                                                 scale=float(SCALES[0]))
                        else:
                            nc.vector.scalar_tensor_tensor(
                                dst, ptiles[j][:, :, :], float(SCALES[j]),
                                dst, ALU.mult, ALU.max)

            def emit_mwh(ib):
                nc.vector.memset(posg[ib][:, 0:16], NEG)
                nc.vector.memset(posg[ib][:, 784:800], NEG)
                m2 = poolp.tile([128, 800], F32, tag="m2", name="m2")
                m4 = poolp.tile([128, 800], F32, tag="m4", name="m4")
                m8 = poolp.tile([128, 800], F32, tag="m8", name="m8")
                a = poolp.tile([128, W], F32, tag="a", name="a")
                g = posg[ib]
                nc.vector.tensor_tensor(m2[:, 0:799], g[:, 0:799], g[:, 1:800],
                                        ALU.max)
                nc.vector.tensor_tensor(m4[:, 0:797], m2[:, 0:797],
                                        m2[:, 2:799], ALU.max)
                nc.vector.tensor_tensor(m8[:, 0:793], m4[:, 0:793],
                                        m4[:, 4:797], ALU.max)
                nc.vector.tensor_tensor(a[:], m8[:, 11:779], m4[:, 18:786],
                                        ALU.max)
                return a

            def emit_at(ib, a):
                for half in range(2):
                    pt = psT.tile([128, 384], F32, tag="pt", name="pt")
                    for k in range(3):
                        c = 3 * half + k
                        nc.tensor.transpose(pt[:, 128 * k:128 * (k + 1)],
                                            a[:, 128 * c:128 * (c + 1)], idt[:])
                    nc.scalar.activation(
                        atg[:, 3 * half:3 * half + 3,
                            16 + 128 * ib:16 + 128 * (ib + 1)],
                        pt[:], AF.Copy)

            def emit_mwv(vib, nvib=1):
                # vertical window max for nvib consecutive 128-col chunks
                av = 16 + 128 * vib
                n = 128 * nvib
                m2 = poolp.tile([128, NB, 272], F32, tag="m2v", name="m2v")
                m4 = poolp.tile([128, NB, 272], F32, tag="m4v", name="m4v")
                m8 = poolp.tile([128, NB, 272], F32, tag="m8v", name="m8v")
                nc.vector.tensor_tensor(m2[:, :, 0:n + 16],
                                        atg[:, :, av - 8:av + n + 8],
                                        atg[:, :, av - 7:av + n + 9], ALU.max)
                nc.vector.tensor_tensor(m4[:, :, 0:n + 14], m2[:, :, 0:n + 14],
                                        m2[:, :, 2:n + 16], ALU.max)
                nc.vector.tensor_tensor(m8[:, :, 2:n + 10], m4[:, :, 2:n + 10],
                                        m4[:, :, 6:n + 14], ALU.max)
                nc.vector.tensor_tensor(ptv[:, :, 128 * vib:128 * vib + n],
                                        m8[:, :, 3:n + 3], m4[:, :, 10:n + 10],
                                        ALU.max)

            def emit_ptt(vib):
                plv = smallp.tile([128, W], F32, tag="plv", name="plv")
                for half in range(2):
                    pt = psT.tile([128, 384], F32, tag="pt", name="pt")
                    for k in range(3):
                        c = 3 * half + k
                        nc.tensor.transpose(
                            pt[:, 128 * k:128 * (k + 1)],
                            ptv[:, c, 128 * vib:128 * (vib + 1)], idt[:])
                    nc.scalar.activation(plv[:, 384 * half:384 * (half + 1)],
                                         pt[:], AF.Copy)
                # fused: mask = (plv max nextafter(0.5)) <= posg
                #   plv >= posg always, so <= means equality (local max) AND
                #   posg > 0.5 via the raised scalar.  One STT instead of
                #   TT(is_equal) + STT(is_gt, and).
                mk = smallp.tile([128, W], U8, tag="mk", name="mk")
                nc.vector.scalar_tensor_tensor(
                    mk[:], plv[:], C0, posg[vib][:, 16:784],
                    ALU.max, ALU.is_le)
                nc.scalar.dma_start(out=MASK[128 * vib:128 * (vib + 1), :],
                                  in_=mk[:])

            alist = {}
            for ib in range(NB):
                emit_wave(ib)
                if ib >= 1:
                    emit_at(ib - 1, alist[ib - 1])
                if ib >= 2:
                    emit_mwv(ib - 2)
                    emit_ptt(ib - 2)
                nc.sync.dma_start(out=CONV[128 * ib:128 * (ib + 1), :],
                                  in_=posg[ib][:, 16:784])
                alist[ib] = emit_mwh(ib)
            emit_at(NB - 1, alist[NB - 1])
            emit_mwv(NB - 2)
            emit_ptt(NB - 2)
            emit_mwv(NB - 1)
            emit_ptt(NB - 1)

    nc.compile()
    return nc


# ---------------------------------------------------------------- host glue
def kernel(C, S, kernel_cos, kernel_sin):
    C = np.asarray(C, dtype=np.float32)
    S = np.asarray(S, dtype=np.float32)
    B = C.shape[0]
    if "nc" not in _CACHE:
        _CACHE["nc"] = _build()
    nc = _CACHE["nc"]
    consts = _consts()
    in_maps = []
    for b in range(B):
        m = _prep_core(C[b, 0], S[b, 0])
        m.update(consts)
        in_maps.append(m)
    res = run_bass_kernel_spmd(nc, in_maps, core_ids=list(range(B)))
    conv = np.stack([r["conv"] for r in res.results])[:, None]
    mask = np.stack([r["mask"] for r in res.results])[:, None].astype(bool)
    return conv.astype(np.float32), mask



# revision 22
# speedup vs baseline: 1.3577x; 1.0010x over previous
"""Trainium2 Bass kernel v2 for nn_Conv1dMultiscaleLocalization.

Per image [768,768], one image per core (B=8 data-parallel):
  resp_j = vconv(C, k_j) + hconv(S, k_j);  conv = max_j resp_j
  pooled = 11x11 max pool; mask = (conv==mw(conv)) & (conv>0.5)

Same numerics as v1 (bf16 hi+lo exact split, fp32 PSUM; 0 mask flips).
Perf changes vs v1 (trace-driven):
  - ldweights=False on consecutive same-stationary matmuls: H conv was
    LDWEIGHTS-gated (600 small matmuls each paying a ~116ns weight load);
    now ~1 load per stationary chunk.  Matmul order restructured (terms and
    halves grouped per stationary) to maximize reuse runs.
  - Batched DMA: c96 stays block-major; st becomes wave-major so each wave's
    H stationaries arrive in one [128,768] transfer; ~30 DMAs vs ~100.
  - mwv (vertical pool window) batched across all 6 column chunks per step
    via one 3D-AP instruction instead of 6, on a single atg tile.
"""
import sys
import numpy as np

sys.path.insert(0, "/opt/trn_rl_repo")

import ml_dtypes  # noqa: E402
import concourse.bacc as bacc  # noqa: E402
import concourse.mybir as mybir  # noqa: E402
import concourse.tile as tile  # noqa: E402
from concourse.bass_utils import run_bass_kernel_spmd  # noqa: E402

F32 = mybir.dt.float32
BF16 = mybir.dt.bfloat16
U8 = mybir.dt.uint8
AF = mybir.ActivationFunctionType
ALU = mybir.AluOpType

H = W = 768
KERNEL_SIZES = [3, 9, 15, 21, 31, 51, 65]
NJ = 7
XJ = [(w - 1) // 2 for w in KERNEL_SIZES]
SCALES = [1.0 / (w - 1) for w in KERNEL_SIZES]
NB = 6          # 128-row blocks per image
NEG = -3.0e38
NTERMS = 2      # bf16 split terms (hi, lo)
# one j per PSUM wave: the combine/evac chain starts as soon as j0's
# matmuls stop instead of after the whole 3-j group (kills the ~12us
# DVE startup bubble seen in the trace); same matmul/op counts.
JG = [(j, j + 1) for j in range(NJ)]
C0 = float(np.nextafter(np.float32(0.5), np.float32(1.0)))  # >0.5 threshold

_CACHE = {}


# ---------------------------------------------------------------- constants
def _sign_band(d, x):
    return np.where((d >= -x) & (d <= -1), 1.0,
                    np.where((d >= 1) & (d <= x), -1.0, 0.0))


def _toeplitz_v2():
    """V stationary [128, NJ*3, 128]: K packs (64 v-rows x 2 terms) by
    partition parity; chunk k covers input rows 128b-32+64k + p//2.
    T2[p, 3j+k, m] = band_j((-32 + 64k + p//2) - m)."""
    T = np.zeros((128, NJ * 3, 128), dtype=np.float32)
    p = np.arange(128)[:, None]
    m = np.arange(128)[None, :]
    for j in range(NJ):
        for k in range(3):
            T[:, 3 * j + k, :] = _sign_band((-32 + 64 * k + p // 2) - m, XJ[j])
    return T


def _band_h2():
    """H moving [128, NJ*128]: K packs (64 w'-cols x 2 terms); chunk a covers
    w' = 64a + p//2, out col w = 64a - 32 + n.
    T2[p, 128j+n] = band_j(p//2 + 32 - n)."""
    T = np.zeros((128, NJ * 128), dtype=np.float32)
    p = np.arange(128)[:, None]
    n = np.arange(128)[None, :]
    for j in range(NJ):
        T[:, 128 * j:128 * (j + 1)] = _sign_band(p // 2 + 32 - n, XJ[j])
    return T


def _split_terms(x):
    terms = []
    r = x
    for _ in range(NTERMS):
        t = r.astype(ml_dtypes.bfloat16)
        terms.append(t)
        r = r - t.astype(np.float32)
    return terms


def _interleave(t0, t1):
    """[R, ...] x2 -> [2R, ...] with rows (2r, 2r+1) = (t0[r], t1[r])."""
    out = np.empty((t0.shape[0] * 2,) + t0.shape[1:], dtype=t0.dtype)
    out[0::2] = t0
    out[1::2] = t1
    return out


def _prep_core(Cb, Sb):
    """c2 [128, NB*3, W]: c2[p, 3b+k, n] = term_{p%2}(C)[128b-32+64k+p//2, n];
    stw2 [128, NB, 12, 128]: stw2[p, ib, a, m] = term_{p%2}(S)[128ib+m,
    64a+p//2]."""
    ct = _split_terms(Cb)
    cint = _interleave(ct[0].astype(np.float32),
                       ct[1].astype(np.float32))  # [1536, W] rows 2v+t
    cpad = np.vstack([np.zeros((64, W), np.float32), cint,
                      np.zeros((192, W), np.float32)])  # row 2(v+32)+t
    c2 = np.zeros((128, NB * 3, W), dtype=np.float32)
    for b in range(NB):
        for k in range(3):
            r0 = 2 * (128 * b + 64 * k)  # = 2*(v0+32) with v0 = 128b-32+64k
            c2[:, 3 * b + k, :] = cpad[r0:r0 + 128, :]
    st = _split_terms(Sb.T)  # [w', u]
    sint = _interleave(st[0].astype(np.float32),
                       st[1].astype(np.float32))  # [1536, u] rows 2w'+t
    stw2 = sint.reshape(12, 128, NB, 128).transpose(1, 2, 0, 3)
    return {"c2": c2.astype(ml_dtypes.bfloat16).reshape(128, -1),
            "stw2": np.ascontiguousarray(stw2).astype(
                ml_dtypes.bfloat16).reshape(128, -1)}


def _consts():
    return {
        "TV2": _toeplitz_v2().astype(ml_dtypes.bfloat16).reshape(128, -1),
        "TH2": _band_h2().astype(ml_dtypes.bfloat16).reshape(128, -1),
        "IDT": np.eye(128, dtype=np.float32),
    }


# ---------------------------------------------------------------- kernel IR
def _build():
    nc = bacc.Bacc()
    C2D = nc.declare_dram_parameter("c2", [128, NB * 3 * W], BF16,
                                    isOutput=False)
    STW2 = nc.declare_dram_parameter("stw2", [128, NB * 12 * 128], BF16,
                                     isOutput=False)
    TV2D = nc.declare_dram_parameter("TV2", [128, NJ * 3 * 128], BF16,
                                     isOutput=False)
    TH2D = nc.declare_dram_parameter("TH2", [128, NJ * 128], BF16,
                                     isOutput=False)
    IDT = nc.declare_dram_parameter("IDT", [128, 128], F32, isOutput=False)
    CONV = nc.declare_dram_parameter("conv", [H, W], F32, isOutput=True)
    MASK = nc.declare_dram_parameter("mask", [H, W], U8, isOutput=True)

    def hspan(a, j):
        # out-col span covered by w'-chunk a (64 cols, both terms)
        return max(0, 64 * a - XJ[j]), min(W, 64 * a + 64 + XJ[j])

    with tile.TileContext(nc) as tc:
        with tc.tile_pool(name="big", bufs=1) as big, \
             tc.tile_pool(name="consts", bufs=1) as cst, \
             tc.tile_pool(name="posg", bufs=1) as posp, \
             tc.tile_pool(name="pool", bufs=2) as poolp, \
             tc.tile_pool(name="atg", bufs=1) as atgp, \
             tc.tile_pool(name="pooled", bufs=1) as pooledp, \
             tc.tile_pool(name="small", bufs=2) as smallp, \
             tc.tile_pool(name="ps", bufs=3, space="PSUM") as ps, \
             tc.tile_pool(name="psT", bufs=2, space="PSUM") as psT:

            c2 = big.tile([128, NB * 3, W], BF16, tag="c2", name="c2")
            stw2 = big.tile([128, NB, 12, 128], BF16, tag="stw2", name="stw2")
            tv2 = cst.tile([128, NJ * 3, 128], BF16, tag="tv2")
            th2 = cst.tile([128, NJ * 128], BF16, tag="th2")
            idt = cst.tile([128, 128], F32, tag="idt")
            # block-priority loads on BOTH hwdge queues: wave ib needs
            # c2[:, 3ib:3ib+3, :] (V) and stw2[:, ib, :, :] (H stationaries).
            # First-wave critical path: TV2 j0-2 slice, then c2 b0 per k-chunk.
            nc.sync.dma_start(out=tv2[:, 0:3, :], in_=TV2D[:, 0:3 * 128])
            nc.scalar.dma_start(out=th2[:], in_=TH2D[:])
            for k in range(3):
                nc.sync.dma_start(out=c2[:, k:k + 1, :],
                                  in_=C2D[:, k * W:(k + 1) * W])
            nc.sync.dma_start(out=tv2[:, 3:21, :], in_=TV2D[:, 3 * 128:])
            nc.scalar.dma_start(out=stw2[:, 0, :, :], in_=STW2[:, 0:12 * 128])
            nc.scalar.dma_start(out=idt[:], in_=IDT[:])
            for b in range(1, NB):
                nc.sync.dma_start(out=c2[:, 3 * b:3 * (b + 1), :],
                                  in_=C2D[:, 3 * b * W:3 * (b + 1) * W])
                nc.scalar.dma_start(
                    out=stw2[:, b, :, :],
                    in_=STW2[:, 12 * 128 * b:12 * 128 * (b + 1)])

            posg = [posp.tile([128, 800], F32, tag=f"posg{ib}", name=f"posg{ib}")
                    for ib in range(NB)]
            atg = atgp.tile([128, NB, 800], F32, tag="atg", name="atg")
            ptv = pooledp.tile([128, NB, W], F32, tag="ptv", name="ptv")
            nc.vector.memset(atg[:, :, 0:16], NEG)
            nc.vector.memset(atg[:, :, 784:800], NEG)

            def last_a(j, h):
                lo_h, hi_h = 384 * h, 384 * (h + 1)
                return max(a for a in range(12)
                           if max(hspan(a, j)[0], lo_h)
                           < min(hspan(a, j)[1], hi_h))

            def emit_wave(ib):
                for (j0, j1) in JG:
                    # 2-bank tiles: [128, 2, 384] padded to [128, 2, 512] so
                    # each half sits bank-aligned; combine reads both at once.
                    ptiles = {j: ps.tile([128, 2, 384], F32, tag="p",
                                         name=f"p{j}",
                                         padded_shape=[128, 2, 512])
                              for j in range(j0, j1)}
                    # ---- V: 3 K-chunks (64 v-rows x 2 terms each) per half
                    for j in range(j0, j1):
                        for k in range(3):
                            for h in range(2):
                                rhs = c2[:, 3 * ib + k, 384 * h:384 * (h + 1)]
                                nc.tensor.matmul(
                                    ptiles[j][:, h, :], tv2[:, 3 * j + k, :],
                                    rhs, start=(k == 0), stop=False,
                                    skip_group_check=True)
                    # ---- H: stationary = stw2 chunk (ib, a); both terms ride
                    spans = {}
                    for a in range(12):
                        for j in range(j0, j1):
                            lo, hi = hspan(a, j)
                            if lo < hi:
                                spans[(a, j)] = (lo, hi)
                    lasts = {}
                    for (a, j) in spans:
                        lasts[j] = a
                    for a in range(12):
                        if not any((a, j) in spans for j in range(j0, j1)):
                            continue
                        lhs = stw2[:, ib, a, :]
                        first = True
                        for j in range(j0, j1):
                            if (a, j) not in spans:
                                continue
                            lo, hi = spans[(a, j)]
                            for h in range(2):
                                l2 = max(lo, 384 * h)
                                h2 = min(hi, 384 * (h + 1))
                                if l2 >= h2:
                                    continue
                                off = 128 * j + (l2 - (64 * a - 32))
                                stop = (lasts[j] == a and h2 == hi)
                                mm = nc.tensor.matmul(
                                    ptiles[j][:, h, l2 - 384 * h:h2 - 384 * h],
                                    lhs, th2[:, off:off + (h2 - l2)],
                                    start=False, stop=stop,
                                    skip_group_check=True)
                                if not first:
                                    mm.ins.ldweights = False
                                first = False
                    # ---- combine into posg, 768 wide via the 2-bank AP
                    dst = posg[ib][:, 16:784]
                    for j in range(j0, j1):
                        if j == 0:
                            nc.scalar.activation(dst, ptiles[0][:, :, :],
                                                 AF.Copy,
                                                 scale=float(SCALES[0]))
                        else:
                            nc.vector.scalar_tensor_tensor(
                                dst, ptiles[j][:, :, :], float(SCALES[j]),
                                dst, ALU.mult, ALU.max)

            def emit_mwh(ib):
                nc.vector.memset(posg[ib][:, 0:16], NEG)
                nc.vector.memset(posg[ib][:, 784:800], NEG)
                m2 = poolp.tile([128, 800], F32, tag="m2", name="m2")
                m4 = poolp.tile([128, 800], F32, tag="m4", name="m4")
                m8 = poolp.tile([128, 800], F32, tag="m8", name="m8")
                a = poolp.tile([128, W], F32, tag="a", name="a")
                g = posg[ib]
                nc.vector.tensor_tensor(m2[:, 0:799], g[:, 0:799], g[:, 1:800],
                                        ALU.max)
                nc.vector.tensor_tensor(m4[:, 0:797], m2[:, 0:797],
                                        m2[:, 2:799], ALU.max)
                nc.vector.tensor_tensor(m8[:, 0:793], m4[:, 0:793],
                                        m4[:, 4:797], ALU.max)
                nc.vector.tensor_tensor(a[:], m8[:, 11:779], m4[:, 18:786],
                                        ALU.max)
                return a

            def emit_at(ib, a):
                for half in range(2):
                    pt = psT.tile([128, 384], F32, tag="pt", name="pt")
                    for k in range(3):
                        c = 3 * half + k
                        nc.tensor.transpose(pt[:, 128 * k:128 * (k + 1)],
                                            a[:, 128 * c:128 * (c + 1)], idt[:])
                    nc.scalar.activation(
                        atg[:, 3 * half:3 * half + 3,
                            16 + 128 * ib:16 + 128 * (ib + 1)],
                        pt[:], AF.Copy)

            def emit_mwv(vib, nvib=1):
                # vertical window max for nvib consecutive 128-col chunks
                av = 16 + 128 * vib
                n = 128 * nvib
                m2 = poolp.tile([128, NB, 272], F32, tag="m2v", name="m2v")
                m4 = poolp.tile([128, NB, 272], F32, tag="m4v", name="m4v")
                m8 = poolp.tile([128, NB, 272], F32, tag="m8v", name="m8v")
                nc.vector.tensor_tensor(m2[:, :, 0:n + 16],
                                        atg[:, :, av - 8:av + n + 8],
                                        atg[:, :, av - 7:av + n + 9], ALU.max)
                nc.vector.tensor_tensor(m4[:, :, 0:n + 14], m2[:, :, 0:n + 14],
                                        m2[:, :, 2:n + 16], ALU.max)
                nc.vector.tensor_tensor(m8[:, :, 2:n + 10], m4[:, :, 2:n + 10],
                                        m4[:, :, 6:n + 14], ALU.max)
                nc.vector.tensor_tensor(ptv[:, :, 128 * vib:128 * vib + n],
                                        m8[:, :, 3:n + 3], m4[:, :, 10:n + 10],
                                        ALU.max)

            def emit_ptt(vib):
                plv = smallp.tile([128, W], F32, tag="plv", name="plv")
                for half in range(2):
                    pt = psT.tile([128, 384], F32, tag="pt", name="pt")
                    for k in range(3):
                        c = 3 * half + k
                        nc.tensor.transpose(
                            pt[:, 128 * k:128 * (k + 1)],
                            ptv[:, c, 128 * vib:128 * (vib + 1)], idt[:])
                    nc.scalar.activation(plv[:, 384 * half:384 * (half + 1)],
                                         pt[:], AF.Copy)
                # fused: mask = (plv max nextafter(0.5)) <= posg
                #   plv >= posg always, so <= means equality (local max) AND
                #   posg > 0.5 via the raised scalar.  One STT instead of
                #   TT(is_equal) + STT(is_gt, and).
                mk = smallp.tile([128, W], U8, tag="mk", name="mk")
                nc.vector.scalar_tensor_tensor(
                    mk[:], plv[:], C0, posg[vib][:, 16:784],
                    ALU.max, ALU.is_le)
                nc.scalar.dma_start(out=MASK[128 * vib:128 * (vib + 1), :],
                                  in_=mk[:])

            alist = {}
            for ib in range(NB):
                emit_wave(ib)
                if ib >= 1:
                    emit_at(ib - 1, alist[ib - 1])
                if ib >= 2:
                    emit_mwv(ib - 2)
                    emit_ptt(ib - 2)
                nc.sync.dma_start(out=CONV[128 * ib:128 * (ib + 1), :],
                                  in_=posg[ib][:, 16:784])
                alist[ib] = emit_mwh(ib)
            emit_at(NB - 1, alist[NB - 1])
            emit_mwv(NB - 2)
            emit_ptt(NB - 2)
            emit_mwv(NB - 1)
            emit_ptt(NB - 1)

    nc.compile()
    return nc


# ---------------------------------------------------------------- host glue
def kernel(C, S, kernel_cos, kernel_sin):
    C = np.asarray(C, dtype=np.float32)
    S = np.asarray(S, dtype=np.float32)
    B = C.shape[0]
    if "nc" not in _CACHE:
        _CACHE["nc"] = _build()
    nc = _CACHE["nc"]
    consts = _consts()
    in_maps = []
    for b in range(B):
        m = _prep_core(C[b, 0], S[b, 0])
        m.update(consts)
        in_maps.append(m)
    res = run_bass_kernel_spmd(nc, in_maps, core_ids=list(range(B)))
    conv = np.stack([r["conv"] for r in res.results])[:, None]
    mask = np.stack([r["mask"] for r in res.results])[:, None].astype(bool)
    return conv.astype(np.float32), mask

